# revision 8
# baseline (speedup 1.0000x reference)
"""Trainium2 Bass kernel for nn_AttentionBlock (GroupNorm + 1x1-conv QKV +
multi-head attention + 1x1-conv proj + residual).

Contract: kernel(**inputs) takes the FULL unsharded inputs (numpy) and
returns the FULL output.  Internally shards data-parallel over batch across
8 NeuronCores (2 samples per core).

v2 design notes (vs the v1 baseline at 548-619us):
  - v1 trace: PE 90% busy but HAM-throttled to 1.2 GHz for 80% of the span
    (attention phase micro-stalls waiting on ScalarE exp); DVE RECIPROCAL on
    [1,512] 1-lane tiles cost 106us; EXP on ScalarE alone cost 176us.
  - exp is split 2:2 per m-chunk between ScalarE (exact EXP) and DVE
    (1-op Schraudolph fast-exp: tensor_scalar f32->int16 round-to-nearest
    emitting bf16 bit patterns; max elem err ~3.3%, softmax-normalization
    cancels most of it).
  - 1/Z: Z rows (partition 64 of the av PSUM tiles) are copied to SBUF rows
    (DVE, 32-aligned partition offsets), DMA-packed to [16,128], one
    reciprocal_approx_fast, DMA-flattened back, gpsimd partition_broadcast,
    and the normalize multiply runs on GpSimd straight out of PSUM.
  - q,k bias+bf16 cast on ScalarE (activation Copy with per-partition bias),
    v^T copies on ScalarE, proj bias folded into the residual source tiles,
    so DVE keeps headroom for the fast-exp stream.
  - ACT table switches batched: both samples' GroupNorm (Sqrt), then both
    samples' qkv/vT (Copy), then attention (Exp).
"""

import math
import os

import numpy as np

import concourse.bacc as bacc
import concourse.tile as tile
from concourse import mybir
from concourse.bass_utils import run_bass_kernel_spmd

F32 = mybir.dt.float32
I16 = mybir.dt.int16
AX = mybir.AxisListType
ALU = mybir.AluOpType
ACT = mybir.ActivationFunctionType

N_CORES = 8
B, C, HH, WW = 16, 512, 32, 32
L = HH * WW            # 1024
BL = B // N_CORES      # batches per core = 2
NH = 8                 # heads
CH = C // NH           # head dim = 64
GROUPS = 32
GS = C // GROUPS       # channels per group = 16
EPS = 1e-5
SCALE2 = 1.0 / math.sqrt(CH)   # combined q*k scale, folded into exp
CT = C // 128          # channel tiles = 4
ST = L // 128          # s-chunks = 8
INV_N = 1.0 / (GS * L)         # group mean divisor
# Schraudolph fast-exp constants (bf16-bit-space, round-to-nearest),
# calibrated for max relative error ~3.3%:
#   i16 = round(p * EXP_A + EXP_B);  bf16_bits(i16) ~= exp(SCALE2 * p)
EXP_A = SCALE2 * math.log2(math.e) * 128.0   # 23.083120654223414
EXP_B = (127.0 - 0.044) * 128.0              # 16250.368

MM_DT = mybir.dt.bfloat16
# how many of the 4 exp tiles per m-chunk go to DVE (rest to ScalarE)
N_DVE_EXP = int(os.environ.get("KERNEL_DVE_EXP", "2"))
STAGE = int(os.environ.get("KERNEL_STAGE", "4"))

LAST_RESULTS = None  # test harness can read exec_time_ns from here


def _build_program():
    nc = bacc.Bacc("TRN2", target_bir_lowering=False, debug=False,
                   num_devices=N_CORES)

    x_d = nc.dram_tensor("x", [BL, C, L], F32, kind="ExternalInput").ap()
    out_d = nc.dram_tensor("out", [BL, C, L], F32, kind="ExternalOutput").ap()
    wqT_d = nc.dram_tensor("wqkvT", [C, 3 * C], MM_DT, kind="ExternalInput").ap()
    wpT_d = nc.dram_tensor("wprojT", [C, C], MM_DT, kind="ExternalInput").ap()
    nw_d = nc.dram_tensor("norm_w", [C], F32, kind="ExternalInput").ap()
    nb_d = nc.dram_tensor("norm_b", [C], F32, kind="ExternalInput").ap()
    qb_d = nc.dram_tensor("qkv_b", [3 * C], F32, kind="ExternalInput").ap()
    pb_d = nc.dram_tensor("proj_b", [C], F32, kind="ExternalInput").ap()
    sel_d = nc.dram_tensor("sel", [CT, 128, GROUPS], F32, kind="ExternalInput").ap()
    fan_d = nc.dram_tensor("fan", [CT, GROUPS, 128], F32, kind="ExternalInput").ap()
    ones_d = nc.dram_tensor("ones", [1, 512], F32, kind="ExternalInput").ap()

    with tile.TileContext(nc) as tc:
        with (
            tc.tile_pool(name="wgt", bufs=1) as wgt,            # persistent weights
            tc.tile_pool(name="xs", bufs=2 * CT) as xs_p,       # raw x (both samples)
            tc.tile_pool(name="xn", bufs=2 * CT) as xn_p,       # normalized x
            tc.tile_pool(name="qk", bufs=4 * CT) as qk_p,       # q,k [c,t] both samples
            tc.tile_pool(name="vt", bufs=2 * ST) as vt_p,       # v^T (+ones)
            tc.tile_pool(name="ew", bufs=8) as ew_p,            # exp(wT) chunks
            tc.tile_pool(name="apool", bufs=CT) as a_p,     # attention out
            tc.tile_pool(name="zz", bufs=2) as z_p,             # z rows / packs
            tc.tile_pool(name="zb", bufs=2) as zb_p,            # 1/Z broadcast
            tc.tile_pool(name="outs", bufs=2) as out_p,         # residual out
            tc.tile_pool(name="tiny", bufs=16) as tiny,         # gn stats etc.
            tc.tile_pool(name="scr", bufs=4) as scr_p,          # bn stats
            tc.tile_pool(name="pmm", bufs=2, space="PSUM") as pmm,   # 2x 2-bank
            tc.tile_pool(name="pa", bufs=2, space="PSUM") as pa_p,   # 2x 2-bank
        ):
            # ---------------- constants / weights ----------------
            wq = []   # qkv_w^T tiles [128 c', 1536 o]
            wp = []   # proj_w^T tiles [128 c', 512 o]
            sel = []
            fan = []
            nw = []
            nb = []
            for i in range(CT):
                w = wgt.tile([128, 3 * C], MM_DT, tag=f"wq{i}")
                nc.sync.dma_start(w[:], wqT_d[128 * i:128 * (i + 1), :])
                wq.append(w)
                w = wgt.tile([128, C], MM_DT, tag=f"wp{i}")
                nc.sync.dma_start(w[:], wpT_d[128 * i:128 * (i + 1), :])
                wp.append(w)
                s = wgt.tile([128, GROUPS], F32, tag=f"sel{i}")
                nc.sync.dma_start(s[:], sel_d[i, :, :])
                sel.append(s)
                f = wgt.tile([GROUPS, 128], F32, tag=f"fan{i}")
                nc.sync.dma_start(f[:], fan_d[i, :, :])
                fan.append(f)
                t = wgt.tile([128, 1], F32, tag=f"nw{i}")
                nc.sync.dma_start(t[:], nw_d[128 * i:128 * (i + 1)].rearrange("(p a) -> p a", a=1))
                nw.append(t)
                t = wgt.tile([128, 1], F32, tag=f"nb{i}")
                nc.sync.dma_start(t[:], nb_d[128 * i:128 * (i + 1)].rearrange("(p a) -> p a", a=1))
                nb.append(t)
            ones_t = wgt.tile([1, 512], F32, tag="ones")
            nc.sync.dma_start(ones_t[:], ones_d[:, :])
            qbv_t = wgt.tile([1, C], F32, tag="qbv")
            nc.sync.dma_start(qbv_t[:], qb_d[2 * C:3 * C].rearrange("(a b) -> a b", a=1))
            qb_qk = []
            for j in range(2 * CT):
                t = wgt.tile([128, 1], F32, tag=f"qb{j}")
                nc.sync.dma_start(t[:], qb_d[128 * j:128 * (j + 1)].rearrange("(p a) -> p a", a=1))
                qb_qk.append(t)
            pb_col = []
            for j in range(CT):
                t = wgt.tile([128, 1], F32, tag=f"pbc{j}")
                nc.sync.dma_start(t[:], pb_d[128 * j:128 * (j + 1)].rearrange("(p a) -> p a", a=1))
                pb_col.append(t)
            eps_t = wgt.tile([GROUPS, 1], F32, tag="eps")
            nc.gpsimd.memset(eps_t[:], EPS)

            # ---- load x for both samples up front ----
            xs = {}
            for b in range(BL):
                for i in range(CT):
                    t = xs_p.tile([128, L], F32, tag="xs", name=f"xs{b}_{i}")
                    nc.sync.dma_start(t[:], x_d[b, 128 * i:128 * (i + 1), :])
                    xs[(b, i)] = t

            # ---------------- GroupNorm for both samples ----------------
            xn = {}
            for b in range(BL):
                stats = []
                for i in range(CT):
                    bns = scr_p.tile([128, 2, 6], F32, tag="bns")
                    xv = xs[(b, i)][:].rearrange("p (s f) -> p s f", f=512)
                    for sgi in range(2):
                        nc.vector.bn_stats(bns[:, sgi, :], xv[:, sgi, :])
                    mv = tiny.tile([128, 2], F32, tag="mv")
                    nc.vector.bn_aggr(mv[:], bns[:])
                    st = tiny.tile([128, 2], F32, tag="stats")
                    # sum = L*mean ; sumsq = L*(var + mean^2)
                    nc.vector.tensor_scalar_mul(st[:, 0:1], mv[:, 0:1], float(L))
                    m2 = tiny.tile([128, 2], F32, tag="m2")
                    nc.vector.tensor_mul(m2[:, 0:1], mv[:, 0:1], mv[:, 0:1])
                    nc.vector.tensor_add(m2[:, 1:2], mv[:, 1:2], m2[:, 0:1])
                    nc.vector.tensor_scalar_mul(st[:, 1:2], m2[:, 1:2], float(L))
                    stats.append(st)
                pg = pmm.tile([128, 1024], F32, tag="mm")
                for i in range(CT):
                    nc.tensor.matmul(pg[0:GROUPS, 0:2], sel[i][:, :], stats[i][:, :],
                                     start=(i == 0), stop=(i == CT - 1))
                # group quantities: mean, E[x2], mean^2, var, std, rstd, mean*rstd
                gq = tiny.tile([GROUPS, 8], F32, tag="gq")
                nc.vector.tensor_scalar_mul(gq[:, 0:1], pg[0:GROUPS, 0:1], INV_N)
                nc.vector.tensor_scalar_mul(gq[:, 1:2], pg[0:GROUPS, 1:2], INV_N)
                nc.vector.tensor_mul(gq[:, 2:3], gq[:, 0:1], gq[:, 0:1])
                nc.vector.tensor_sub(gq[:, 3:4], gq[:, 1:2], gq[:, 2:3])
                nc.scalar.activation(gq[:, 4:5], gq[:, 3:4], ACT.Sqrt,
                                     bias=eps_t[:])
                nc.vector.reciprocal(gq[:, 5:6], gq[:, 4:5])
                nc.vector.tensor_mul(gq[:, 6:7], gq[:, 0:1], gq[:, 5:6])
                # fan out to channels, make per-channel scale/bias
                for i in range(CT):
                    pf = pmm.tile([128, 1024], F32, tag="mm")
                    nc.tensor.matmul(pf[0:128, 0:2], fan[i][:, :], gq[:, 5:7],
                                     start=True, stop=True)
                    scb = tiny.tile([128, 3], F32, tag="scb")
                    nc.vector.tensor_mul(scb[:, 0:1], pf[0:128, 0:1], nw[i][:])
                    nc.vector.tensor_mul(scb[:, 1:2], pf[0:128, 1:2], nw[i][:])
                    nc.vector.tensor_sub(scb[:, 2:3], nb[i][:], scb[:, 1:2])
                    t = xn_p.tile([128, L], MM_DT, tag="xn", name=f"xn{b}_{i}")
                    nc.vector.tensor_scalar(t[:], xs[(b, i)][:], scb[:, 0:1],
                                            scb[:, 2:3], op0=ALU.mult, op1=ALU.add)
                    xn[(b, i)] = t
                    # fold proj bias into the residual source (in place)
                    nc.vector.tensor_scalar_add(xs[(b, i)][:], xs[(b, i)][:],
                                                pb_col[i][:])

            if STAGE == 1:
                for b in range(BL):
                    for i in range(CT):
                        nc.sync.dma_start(out_d[b, 128 * i:128 * (i + 1), :],
                                          xn[(b, i)][:].bitcast(F32))
                nc.compile()
                return nc

            # ---------------- QKV + v^T for both samples ----------------
            qk = {}
            vt = {}
            for b in range(BL):
                # q,k as [c, t]; bias-add + bf16 cast on ScalarE (Copy)
                for j in range(2 * CT):
                    t = qk_p.tile([128, L], MM_DT, tag="qk", name=f"qk{b}_{j}")
                    pq = pmm.tile([128, 1024], F32, tag="mm")
                    for n in range(2):
                        nsl = slice(512 * n, 512 * (n + 1))
                        for i in range(CT):
                            nc.tensor.matmul(
                                pq[:, nsl],
                                wq[i][:, 128 * j:128 * (j + 1)],
                                xn[(b, i)][:, nsl],
                                start=(i == 0), stop=(i == CT - 1))
                    nc.scalar.activation(t[:, :], pq[:, :], ACT.Identity,
                                         bias=qb_qk[j][:])
                    qk[(b, j)] = t

                # v^T [s, c] with a ones column per head (emits Z in the
                # av matmul); ones cols set by gpsimd memset per tile
                for m in range(ST):
                    pvf = pmm.tile([128, 1024], F32, tag="mm")
                    pv = pvf[:, 0:512]
                    for i in range(CT):
                        nc.tensor.matmul(
                            pv[:, :],
                            xn[(b, i)][:, 128 * m:128 * (m + 1)],
                            wq[i][:, 2 * C:3 * C],
                            start=(i == 0), stop=False)
                    nc.tensor.matmul(pv[:, :], ones_t[0:1, 0:128],
                                     qbv_t[0:1, :], start=False, stop=True)
                    t = vt_p.tile([128, NH * (CH + 1)], MM_DT, tag="vt",
                                  name=f"vt{b}_{m}")
                    dst = t[:].rearrange("p (h x) -> p h x", x=CH + 1)
                    nc.scalar.activation(
                        dst[:, :, 0:CH],
                        pv.rearrange("p (h x) -> p h x", x=CH),
                        ACT.Copy)
                    nc.gpsimd.memset(dst[:, :, CH:CH + 1], 1.0)
                    vt[(b, m)] = t

            if STAGE == 2:
                for b in range(BL):
                    for i in range(CT):
                        nc.sync.dma_start(out_d[b, 128 * i:128 * (i + 1), :],
                                          qk[(b, i)][:].bitcast(F32))
                nc.compile()
                return nc

            # ---------------- attention + proj per sample ----------------
            for b in range(BL):
                a_tiles = []
                for hp in range(NH // 2):
                    q_t = qk[(b, hp)]
                    k_t = qk[(b, CT + hp)]
                    # one fused [CH+1, 1024] psum per hh (2 banks each)
                    pa_h = [pa_p.tile([CH + 1, L], F32, tag="pa",
                                      name=f"pa{b}_{hp}_{hh}") for hh in range(2)]
                    prev = None
                    for m in range(ST):
                        msl = slice(128 * m, 128 * (m + 1))
                        # scores: one fused [128,1024] psum per hh
                        pw = []
                        for hh, (plo, phi, tp) in enumerate(
                                ((0, CH, (0, 0)), (CH, 128, (CH, 0)))):
                            p = pmm.tile([128, 1024], F32, tag="mm")
                            for n in range(2):
                                nsl = slice(512 * n, 512 * (n + 1))
                                nc.tensor.matmul(p[:, nsl],
                                                 k_t[plo:phi, msl],
                                                 q_t[plo:phi, nsl],
                                                 start=True, stop=True,
                                                 tile_position=tp)
                            pw.append((hh, p))
                        # exp: one DVE fast-exp + one ScalarE exact exp per m;
                        # alternate the engine across m for error mixing
                        ews = []
                        for hh, p in pw:
                            if (hh + m) % 2 == 0:
                                e = ew_p.tile([128, L], I16, tag="ewi")
                                nc.vector.tensor_scalar(e[:], p[:, :], EXP_A,
                                                        EXP_B, op0=ALU.mult,
                                                        op1=ALU.add)
                                ews.append((hh, e.bitcast(MM_DT)))
                            else:
                                e = ew_p.tile([128, L], MM_DT, tag="ewa")
                                nc.scalar.activation(e[:], p[:, :], ACT.Exp,
                                                     scale=SCALE2)
                                ews.append((hh, e[:]))
                        if prev is not None:
                            pm, pews = prev
                            for hh, e in pews:
                                lhs = vt[(b, pm)][:, (2 * hp + hh) * (CH + 1):
                                                  (2 * hp + hh + 1) * (CH + 1)]
                                for n in range(2):
                                    nsl = slice(512 * n, 512 * (n + 1))
                                    nc.tensor.matmul(pa_h[hh][:, nsl], lhs,
                                                     e[:, nsl],
                                                     start=(pm == 0), stop=False)
                        prev = (m, ews)
                    pm, pews = prev
                    for hh, e in pews:
                        lhs = vt[(b, pm)][:, (2 * hp + hh) * (CH + 1):
                                          (2 * hp + hh + 1) * (CH + 1)]
                        for n in range(2):
                            nsl = slice(512 * n, 512 * (n + 1))
                            nc.tensor.matmul(pa_h[hh][:, nsl], lhs, e[:, nsl],
                                             start=False, stop=True)

                    # ---- normalize: 1/Z on packed partitions ----
                    # Z rows (partition CH of pa tiles) -> SBUF rows
                    zrow = []
                    for hh in range(2):
                        zr = z_p.tile([1, L], F32, tag="zrow",
                                      name=f"zrow{b}_{hp}_{hh}")
                        nc.vector.tensor_copy(zr[:, :], pa_h[hh][CH:CH + 1, :])
                        zrow.append(zr)
                    # pack [2x(1,1024)] -> [16,128], one recip, flatten back
                    zp = z_p.tile([16, 128], F32, tag="zp",
                                  name=f"zp{b}_{hp}")
                    nc.sync.dma_start(zp[0:8, :], zrow[0][0:1, :])
                    nc.sync.dma_start(zp[8:16, :], zrow[1][0:1, :])
                    zrp = z_p.tile([16, 128], F32, tag="zrp",
                                   name=f"zrp{b}_{hp}")
                    nc.vector.reciprocal_approx_fast(zrp[:], zp[:])
                    a_t = a_p.tile([128, L], MM_DT, tag="a", name=f"a{b}_{hp}")
                    for hh in range(2):
                        rzf = z_p.tile([1, L], F32, tag="rzf",
                                       name=f"rzf{b}_{hp}_{hh}")
                        nc.sync.dma_start(rzf[0:1, :], zrp[8 * hh:8 * (hh + 1), :])
                        rzb = zb_p.tile([CH, L], F32, tag="zb")
                        nc.gpsimd.partition_broadcast(rzb[:], rzf[:])
                        rows = slice(CH * hh, CH * (hh + 1))
                        nc.vector.tensor_mul(a_t[rows, :], pa_h[hh][0:CH, :],
                                             rzb[:, :])
                    a_tiles.append(a_t)

                if STAGE == 3:
                    for i in range(CT):
                        nc.sync.dma_start(out_d[b, 128 * i:128 * (i + 1), :],
                                          a_tiles[i][:].bitcast(F32))
                    continue

                # ---- proj + residual (proj bias pre-folded into xs) ----
                for j in range(CT):
                    o_t = out_p.tile([128, L], F32, tag="o")
                    pp = pmm.tile([128, 1024], F32, tag="mm")
                    for n in range(2):
                        nsl = slice(512 * n, 512 * (n + 1))
                        for i in range(CT):
                            nc.tensor.matmul(
                                pp[:, nsl],
                                wp[i][:, 128 * j:128 * (j + 1)],
                                a_tiles[i][:, nsl],
                                start=(i == 0), stop=(i == CT - 1))
                    nc.vector.tensor_add(o_t[:, :], pp[:, :], xs[(b, j)][:, :])
                    nc.sync.dma_start(out_d[b, 128 * j:128 * (j + 1), :], o_t[:])

    nc.compile()
    return nc


_prog_cache = {}


def _get_program():
    key = (STAGE, N_DVE_EXP)
    if key not in _prog_cache:
        _prog_cache[key] = _build_program()
    return _prog_cache[key]


def _host_constants():
    # group selector: sel[i][p, g] = 1 where global group of (tile i, part p)
    # is g;  fan[i][g, p] = same, transposed (for the fan-out matmul lhsT).
    sel = np.zeros((CT, 128, GROUPS), dtype=np.float32)
    fan = np.zeros((CT, GROUPS, 128), dtype=np.float32)
    for i in range(CT):
        for p in range(128):
            g = (128 * i + p) // GS
            sel[i, p, g] = 1.0
            fan[i, g, p] = 1.0
    ones = np.ones((1, 512), dtype=np.float32)
    return sel, fan, ones


def kernel(x, norm_w, norm_b, qkv_w, qkv_b, proj_w, proj_b):
    global LAST_RESULTS
    x = np.ascontiguousarray(np.asarray(x, dtype=np.float32))
    np_mm = mybir.dt.np(MM_DT)
    wqkvT = np.ascontiguousarray(np.asarray(qkv_w, dtype=np.float32).T.astype(np_mm))
    wprojT = np.ascontiguousarray(np.asarray(proj_w, dtype=np.float32).T.astype(np_mm))
    sel, fan, ones = _host_constants()

    xr = x.reshape(B, C, L)
    nc = _get_program()

    common = {
        "wqkvT": wqkvT,
        "wprojT": wprojT,
        "norm_w": np.ascontiguousarray(norm_w, dtype=np.float32),
        "norm_b": np.ascontiguousarray(norm_b, dtype=np.float32),
        "qkv_b": np.ascontiguousarray(qkv_b, dtype=np.float32),
        "proj_b": np.ascontiguousarray(proj_b, dtype=np.float32),
        "sel": sel,
        "fan": fan,
        "ones": ones,
    }
    in_maps = []
    for c in range(N_CORES):
        m = dict(common)
        m["x"] = np.ascontiguousarray(xr[BL * c:BL * (c + 1)])
        in_maps.append(m)

    trace = os.environ.get("KERNEL_TRACE", "0") == "1"
    kwargs = {}
    if trace:
        kwargs = dict(trace=True, trace_cores=[0])
    res = run_bass_kernel_spmd(nc, in_maps, core_ids=list(range(N_CORES)),
                               **kwargs)
    LAST_RESULTS = res
    out = np.concatenate([res.results[c]["out"] for c in range(N_CORES)], axis=0)
    return out.reshape(B, C, HH, WW)


# revision 10
# speedup vs baseline: 1.0811x; 1.0811x over previous
"""Trainium2 Bass kernel for nn_AttentionBlock (GroupNorm + 1x1-conv QKV +
multi-head attention + 1x1-conv proj + residual).

Contract: kernel(**inputs) takes the FULL unsharded inputs (numpy) and
returns the FULL output.  Internally shards data-parallel over batch across
8 NeuronCores (2 samples per core).

v4 design (v1 baseline 548us -> v2 411us -> v4):
  - PE is the roofline engine (~200K cycles/sample).  The enemy is the HAM
    clock gate: any recurring PE micro-stall (waiting on exp or PSUM-slot
    reuse) re-throttles the PE to 1.2 GHz.  So the attention inner loop is
    paced such that exp always finishes well before its PSUM slot is needed:
    attention(sample0) is INTERLEAVED with qkv/vT(sample1) at head-pair
    granularity, and attention(sample1) with proj(sample0).  The extra PE
    work between slot reuses (~2x) gives the exp engines ample slack.
  - exp split per m-chunk: one fused [128,1024] DVE Schraudolph fast-exp
    (tensor_scalar f32->int16 round-to-nearest = bf16 bits of exp; max elem
    err ~3.3%, cancelled by softmax normalization) + one fused ScalarE
    exact Exp.  Z rides along as a ones-column in the v^T stationary.
  - 1/Z: Z rows copied out of PSUM on ScalarE, DMA-packed to [16,128],
    one DVE reciprocal_approx_fast per head-pair, DMA-flattened, gpsimd
    partition_broadcast, DVE multiply.
  - qkv bias+bf16 cast: single fused ScalarE Identity per output chunk.
    proj bias pre-folded into the residual source tiles (DVE, gn phase).
"""

import math
import os

import numpy as np

import concourse.bacc as bacc
import concourse.tile as tile
from concourse import mybir
from concourse.bass_utils import run_bass_kernel_spmd

F32 = mybir.dt.float32
I16 = mybir.dt.int16
ALU = mybir.AluOpType
ACT = mybir.ActivationFunctionType

N_CORES = 8
B, C, HH, WW = 16, 512, 32, 32
L = HH * WW            # 1024
BL = B // N_CORES      # batches per core = 2
NH = 8                 # heads
CH = C // NH           # head dim = 64
GROUPS = 32
GS = C // GROUPS       # channels per group = 16
EPS = 1e-5
SCALE2 = 1.0 / math.sqrt(CH)   # combined q*k scale, folded into exp
CT = C // 128          # channel tiles = 4
ST = L // 128          # s-chunks = 8
INV_N = 1.0 / (GS * L)         # group mean divisor
# Schraudolph fast-exp constants (bf16-bit-space, round-to-nearest):
#   i16 = round(p * EXP_A + EXP_B);  bf16_bits(i16) ~= exp(SCALE2 * p)
EXP_A = SCALE2 * math.log2(math.e) * 128.0   # 23.083120654223414
EXP_B = (127.0 - 0.044) * 128.0              # 16250.368

MM_DT = mybir.dt.bfloat16

LAST_RESULTS = None  # test harness can read exec_time_ns from here


def _build_program():
    nc = bacc.Bacc("TRN2", target_bir_lowering=False, debug=False,
                   num_devices=N_CORES)

    x_d = nc.dram_tensor("x", [BL, C, L], F32, kind="ExternalInput").ap()
    out_d = nc.dram_tensor("out", [BL, C, L], F32, kind="ExternalOutput").ap()
    wqT_d = nc.dram_tensor("wqkvT", [C, 3 * C], MM_DT, kind="ExternalInput").ap()
    wpT_d = nc.dram_tensor("wprojT", [C, C], MM_DT, kind="ExternalInput").ap()
    nw_d = nc.dram_tensor("norm_w", [C], F32, kind="ExternalInput").ap()
    nb_d = nc.dram_tensor("norm_b", [C], F32, kind="ExternalInput").ap()
    qb_d = nc.dram_tensor("qkv_b", [3 * C], F32, kind="ExternalInput").ap()
    pb_d = nc.dram_tensor("proj_b", [C], F32, kind="ExternalInput").ap()
    sel_d = nc.dram_tensor("sel", [CT, 128, GROUPS], F32, kind="ExternalInput").ap()
    fan_d = nc.dram_tensor("fan", [CT, GROUPS, 128], F32, kind="ExternalInput").ap()
    ones_d = nc.dram_tensor("ones", [1, 512], F32, kind="ExternalInput").ap()

    with tile.TileContext(nc) as tc:
        with (
            tc.tile_pool(name="wgt", bufs=1) as wgt,
            tc.tile_pool(name="xs", bufs=2 * CT) as xs_p,
            tc.tile_pool(name="xn", bufs=2 * CT) as xn_p,
            tc.tile_pool(name="qk", bufs=4 * CT) as qk_p,
            tc.tile_pool(name="vt", bufs=2 * ST) as vt_p,
            tc.tile_pool(name="ew", bufs=6) as ew_p,
            tc.tile_pool(name="apool", bufs=2 * CT) as a_p,
            tc.tile_pool(name="zz", bufs=2) as z_p,
            tc.tile_pool(name="zb", bufs=2) as zb_p,
            tc.tile_pool(name="outs", bufs=2) as out_p,
            tc.tile_pool(name="tiny", bufs=16) as tiny,
            tc.tile_pool(name="scr", bufs=4) as scr_p,
            tc.tile_pool(name="pmm", bufs=2, space="PSUM") as pmm,   # 2x 2-bank
            tc.tile_pool(name="pa", bufs=2, space="PSUM") as pa_p,   # 2x 2-bank
        ):
            # ---------------- constants / weights ----------------
            wq, wp, sel, fan, nw, nb = [], [], [], [], [], []
            for i in range(CT):
                w = wgt.tile([128, 3 * C], MM_DT, tag=f"wq{i}")
                nc.sync.dma_start(w[:], wqT_d[128 * i:128 * (i + 1), :])
                wq.append(w)
                w = wgt.tile([128, C], MM_DT, tag=f"wp{i}")
                nc.sync.dma_start(w[:], wpT_d[128 * i:128 * (i + 1), :])
                wp.append(w)
                s = wgt.tile([128, GROUPS], F32, tag=f"sel{i}")
                nc.sync.dma_start(s[:], sel_d[i, :, :])
                sel.append(s)
                f = wgt.tile([GROUPS, 128], F32, tag=f"fan{i}")
                nc.sync.dma_start(f[:], fan_d[i, :, :])
                fan.append(f)
                t = wgt.tile([128, 1], F32, tag=f"nw{i}")
                nc.sync.dma_start(t[:], nw_d[128 * i:128 * (i + 1)].rearrange("(p a) -> p a", a=1))
                nw.append(t)
                t = wgt.tile([128, 1], F32, tag=f"nb{i}")
                nc.sync.dma_start(t[:], nb_d[128 * i:128 * (i + 1)].rearrange("(p a) -> p a", a=1))
                nb.append(t)
            ones_t = wgt.tile([1, 512], F32, tag="ones")
            nc.sync.dma_start(ones_t[:], ones_d[:, :])
            qbv_t = wgt.tile([1, C], F32, tag="qbv")
            nc.sync.dma_start(qbv_t[:], qb_d[2 * C:3 * C].rearrange("(a b) -> a b", a=1))
            qb_qk = []
            for j in range(2 * CT):
                t = wgt.tile([128, 1], F32, tag=f"qb{j}")
                nc.sync.dma_start(t[:], qb_d[128 * j:128 * (j + 1)].rearrange("(p a) -> p a", a=1))
                qb_qk.append(t)
            pb_col = []
            for j in range(CT):
                t = wgt.tile([128, 1], F32, tag=f"pbc{j}")
                nc.sync.dma_start(t[:], pb_d[128 * j:128 * (j + 1)].rearrange("(p a) -> p a", a=1))
                pb_col.append(t)
            eps_t = wgt.tile([GROUPS, 1], F32, tag="eps")
            nc.gpsimd.memset(eps_t[:], EPS)

            # ---- load x for both samples up front ----
            xs = {}
            for b in range(BL):
                for i in range(CT):
                    t = xs_p.tile([128, L], F32, tag="xs", name=f"xs{b}_{i}")
                    nc.sync.dma_start(t[:], x_d[b, 128 * i:128 * (i + 1), :])
                    xs[(b, i)] = t

            xn = {}
            qk = {}
            vt = {}
            a_tiles = {}

            def gn(b):
                stats = []
                for i in range(CT):
                    bns = scr_p.tile([128, 2, 6], F32, tag="bns", name=f"bns{b}_{i}")
                    xv = xs[(b, i)][:].rearrange("p (s f) -> p s f", f=512)
                    for sgi in range(2):
                        nc.vector.bn_stats(bns[:, sgi, :], xv[:, sgi, :])
                    mv = tiny.tile([128, 2], F32, tag="mv", name=f"mv{b}_{i}")
                    nc.vector.bn_aggr(mv[:], bns[:])
                    st = tiny.tile([128, 2], F32, tag="stats", name=f"st{b}_{i}")
                    nc.vector.tensor_scalar_mul(st[:, 0:1], mv[:, 0:1], float(L))
                    m2 = tiny.tile([128, 2], F32, tag="m2", name=f"m2{b}_{i}")
                    nc.vector.tensor_mul(m2[:, 0:1], mv[:, 0:1], mv[:, 0:1])
                    nc.vector.tensor_add(m2[:, 1:2], mv[:, 1:2], m2[:, 0:1])
                    nc.vector.tensor_scalar_mul(st[:, 1:2], m2[:, 1:2], float(L))
                    stats.append(st)
                pg = pmm.tile([128, 1024], F32, tag="mm", name=f"pg{b}")
                for i in range(CT):
                    nc.tensor.matmul(pg[0:GROUPS, 0:2], sel[i][:, :], stats[i][:, :],
                                     start=(i == 0), stop=(i == CT - 1))
                gq = tiny.tile([GROUPS, 8], F32, tag="gq", name=f"gq{b}")
                nc.vector.tensor_scalar_mul(gq[:, 0:1], pg[0:GROUPS, 0:1], INV_N)
                nc.vector.tensor_scalar_mul(gq[:, 1:2], pg[0:GROUPS, 1:2], INV_N)
                nc.vector.tensor_mul(gq[:, 2:3], gq[:, 0:1], gq[:, 0:1])
                nc.vector.tensor_sub(gq[:, 3:4], gq[:, 1:2], gq[:, 2:3])
                nc.scalar.activation(gq[:, 4:5], gq[:, 3:4], ACT.Sqrt,
                                     bias=eps_t[:])
                nc.vector.reciprocal(gq[:, 5:6], gq[:, 4:5])
                nc.vector.tensor_mul(gq[:, 6:7], gq[:, 0:1], gq[:, 5:6])
                for i in range(CT):
                    pf = pmm.tile([128, 1024], F32, tag="mm", name=f"pf{b}_{i}")
                    nc.tensor.matmul(pf[0:128, 0:2], fan[i][:, :], gq[:, 5:7],
                                     start=True, stop=True)
                    scb = tiny.tile([128, 3], F32, tag="scb", name=f"scb{b}_{i}")
                    nc.vector.tensor_mul(scb[:, 0:1], pf[0:128, 0:1], nw[i][:])
                    nc.vector.tensor_mul(scb[:, 1:2], pf[0:128, 1:2], nw[i][:])
                    nc.vector.tensor_sub(scb[:, 2:3], nb[i][:], scb[:, 1:2])
                    t = xn_p.tile([128, L], MM_DT, tag="xn", name=f"xn{b}_{i}")
                    nc.vector.tensor_scalar(t[:], xs[(b, i)][:], scb[:, 0:1],
                                            scb[:, 2:3], op0=ALU.mult, op1=ALU.add)
                    xn[(b, i)] = t
                    # fold proj bias into the residual source (in place)
                    nc.vector.tensor_scalar_add(xs[(b, i)][:], xs[(b, i)][:],
                                                pb_col[i][:])

            def qkv_j(b, j):
                t = qk_p.tile([128, L], MM_DT, tag="qk", name=f"qk{b}_{j}")
                pq = pmm.tile([128, 1024], F32, tag="mm", name=f"pq{b}_{j}")
                for n in range(2):
                    nsl = slice(512 * n, 512 * (n + 1))
                    for i in range(CT):
                        nc.tensor.matmul(pq[:, nsl],
                                         wq[i][:, 128 * j:128 * (j + 1)],
                                         xn[(b, i)][:, nsl],
                                         start=(i == 0), stop=(i == CT - 1))
                nc.scalar.activation(t[:, :], pq[:, :], ACT.Identity,
                                     bias=qb_qk[j][:])
                qk[(b, j)] = t

            def vt_m(b, m):
                pvf = pmm.tile([128, 1024], F32, tag="mm", name=f"pv{b}_{m}")
                pv = pvf[:, 0:512]
                for i in range(CT):
                    nc.tensor.matmul(pv,
                                     xn[(b, i)][:, 128 * m:128 * (m + 1)],
                                     wq[i][:, 2 * C:3 * C],
                                     start=(i == 0), stop=False)
                nc.tensor.matmul(pv, ones_t[0:1, 0:128], qbv_t[0:1, :],
                                 start=False, stop=True)
                t = vt_p.tile([128, NH * (CH + 1)], MM_DT, tag="vt",
                              name=f"vt{b}_{m}")
                dst = t[:].rearrange("p (h x) -> p h x", x=CH + 1)
                nc.scalar.activation(dst[:, :, 0:CH],
                                     pv.rearrange("p (h x) -> p h x", x=CH),
                                     ACT.Copy)
                nc.gpsimd.memset(dst[:, :, CH:CH + 1], 1.0)
                vt[(b, m)] = t

            def attn_hp(b, hp):
                q_t = qk[(b, hp)]
                k_t = qk[(b, CT + hp)]
                pa_h = [pa_p.tile([CH + 1, L], F32, tag="pa",
                                  name=f"pa{b}_{hp}_{hh}") for hh in range(2)]
                prev = None
                for m in range(ST):
                    msl = slice(128 * m, 128 * (m + 1))
                    pw = []
                    for hh, (plo, phi, tp) in enumerate(
                            ((0, CH, (0, 0)), (CH, 128, (CH, 0)))):
                        p = pmm.tile([128, 1024], F32, tag="mm",
                                     name=f"ps{b}_{hp}_{m}_{hh}")
                        for n in range(2):
                            nsl = slice(512 * n, 512 * (n + 1))
                            nc.tensor.matmul(p[:, nsl], k_t[plo:phi, msl],
                                             q_t[plo:phi, nsl],
                                             start=True, stop=True,
                                             tile_position=tp)
                        pw.append((hh, p))
                    ews = []
                    for hh, p in pw:
                        if (hh + m) % 2 == 0:
                            e = ew_p.tile([128, L], I16, tag="ewi")
                            nc.vector.tensor_scalar(e[:], p[:, :], EXP_A, EXP_B,
                                                    op0=ALU.mult, op1=ALU.add)
                            ews.append((hh, e.bitcast(MM_DT)))
                        else:
                            e = ew_p.tile([128, L], MM_DT, tag="ewa")
                            nc.scalar.activation(e[:], p[:, :], ACT.Exp,
                                                 scale=SCALE2)
                            ews.append((hh, e[:]))
                    if prev is not None:
                        pm, pews = prev
                        for hh, e in pews:
                            lhs = vt[(b, pm)][:, (2 * hp + hh) * (CH + 1):
                                              (2 * hp + hh + 1) * (CH + 1)]
                            for n in range(2):
                                nsl = slice(512 * n, 512 * (n + 1))
                                nc.tensor.matmul(pa_h[hh][:, nsl], lhs, e[:, nsl],
                                                 start=(pm == 0), stop=False)
                    prev = (m, ews)
                pm, pews = prev
                for hh, e in pews:
                    lhs = vt[(b, pm)][:, (2 * hp + hh) * (CH + 1):
                                      (2 * hp + hh + 1) * (CH + 1)]
                    for n in range(2):
                        nsl = slice(512 * n, 512 * (n + 1))
                        nc.tensor.matmul(pa_h[hh][:, nsl], lhs, e[:, nsl],
                                         start=False, stop=True)

                # ---- normalize ----
                zrow = []
                for hh in range(2):
                    zr = z_p.tile([1, L], F32, tag="zrow",
                                  name=f"zrow{b}_{hp}_{hh}")
                    nc.scalar.activation(zr[:, :], pa_h[hh][CH:CH + 1, :],
                                         ACT.Copy)
                    zrow.append(zr)
                zp = z_p.tile([16, 128], F32, tag="zp", name=f"zp{b}_{hp}")
                nc.sync.dma_start(zp[0:8, :], zrow[0][0:1, :])
                nc.sync.dma_start(zp[8:16, :], zrow[1][0:1, :])
                zrp = z_p.tile([16, 128], F32, tag="zrp", name=f"zrp{b}_{hp}")
                nc.vector.reciprocal_approx_fast(zrp[:], zp[:])
                a_t = a_p.tile([128, L], MM_DT, tag="a", name=f"a{b}_{hp}")
                for hh in range(2):
                    rzf = z_p.tile([1, L], F32, tag="rzf",
                                   name=f"rzf{b}_{hp}_{hh}")
                    nc.sync.dma_start(rzf[0:1, :], zrp[8 * hh:8 * (hh + 1), :])
                    rzb = zb_p.tile([CH, L], F32, tag="zb")
                    nc.gpsimd.partition_broadcast(rzb[:], rzf[:])
                    rows = slice(CH * hh, CH * (hh + 1))
                    nc.vector.tensor_mul(a_t[rows, :], pa_h[hh][0:CH, :],
                                         rzb[:, :])
                a_tiles[(b, hp)] = a_t

            def proj_j(b, j):
                o_t = out_p.tile([128, L], F32, tag="o", name=f"o{b}_{j}")
                pp = pmm.tile([128, 1024], F32, tag="mm", name=f"pp{b}_{j}")
                for n in range(2):
                    nsl = slice(512 * n, 512 * (n + 1))
                    for i in range(CT):
                        nc.tensor.matmul(pp[:, nsl],
                                         wp[i][:, 128 * j:128 * (j + 1)],
                                         a_tiles[(b, i)][:, nsl],
                                         start=(i == 0), stop=(i == CT - 1))
                nc.vector.tensor_add(o_t[:, :], pp[:, :], xs[(b, j)][:, :])
                nc.sync.dma_start(out_d[b, 128 * j:128 * (j + 1), :], o_t[:])

            # ---------------- schedule ----------------
            gn(0)
            gn(1)
            for j in range(2 * CT):
                qkv_j(0, j)
            for m in range(ST):
                vt_m(0, m)
            # attention(0) interleaved with qkv/vT of sample 1
            fill1 = ([("qkv", 1, j) for j in range(2 * CT)]
                     + [("vt", 1, m) for m in range(ST)])
            for hp in range(NH // 2):
                attn_hp(0, hp)
                for _ in range(4):
                    kind, bb, idx = fill1.pop(0)
                    (qkv_j if kind == "qkv" else vt_m)(bb, idx)
            # attention(1) interleaved with proj of sample 0
            for hp in range(NH // 2):
                attn_hp(1, hp)
                proj_j(0, hp)
            for j in range(CT):
                proj_j(1, j)

    nc.compile()
    return nc


_prog_cache = {}


def _get_program():
    if "p" not in _prog_cache:
        _prog_cache["p"] = _build_program()
    return _prog_cache["p"]


def _host_constants():
    sel = np.zeros((CT, 128, GROUPS), dtype=np.float32)
    fan = np.zeros((CT, GROUPS, 128), dtype=np.float32)
    for i in range(CT):
        for p in range(128):
            g = (128 * i + p) // GS
            sel[i, p, g] = 1.0
            fan[i, g, p] = 1.0
    ones = np.ones((1, 512), dtype=np.float32)
    return sel, fan, ones


def kernel(x, norm_w, norm_b, qkv_w, qkv_b, proj_w, proj_b):
    global LAST_RESULTS
    x = np.ascontiguousarray(np.asarray(x, dtype=np.float32))
    np_mm = mybir.dt.np(MM_DT)
    wqkvT = np.ascontiguousarray(np.asarray(qkv_w, dtype=np.float32).T.astype(np_mm))
    wprojT = np.ascontiguousarray(np.asarray(proj_w, dtype=np.float32).T.astype(np_mm))
    sel, fan, ones = _host_constants()

    xr = x.reshape(B, C, L)
    nc = _get_program()

    common = {
        "wqkvT": wqkvT,
        "wprojT": wprojT,
        "norm_w": np.ascontiguousarray(norm_w, dtype=np.float32),
        "norm_b": np.ascontiguousarray(norm_b, dtype=np.float32),
        "qkv_b": np.ascontiguousarray(qkv_b, dtype=np.float32),
        "proj_b": np.ascontiguousarray(proj_b, dtype=np.float32),
        "sel": sel,
        "fan": fan,
        "ones": ones,
    }
    in_maps = []
    for c in range(N_CORES):
        m = dict(common)
        m["x"] = np.ascontiguousarray(xr[BL * c:BL * (c + 1)])
        in_maps.append(m)

    trace = os.environ.get("KERNEL_TRACE", "0") == "1"
    kwargs = {}
    if trace:
        kwargs = dict(trace=True, trace_cores=[0])
    res = run_bass_kernel_spmd(nc, in_maps, core_ids=list(range(N_CORES)),
                               **kwargs)
    LAST_RESULTS = res
    out = np.concatenate([res.results[c]["out"] for c in range(N_CORES)], axis=0)
    return out.reshape(B, C, HH, WW)


# revision 11
# speedup vs baseline: 1.3951x; 1.2904x over previous
"""Trainium2 Bass kernel for nn_AttentionBlock (GroupNorm + 1x1-conv QKV +
multi-head attention + 1x1-conv proj + residual).

Contract: kernel(**inputs) takes the FULL unsharded inputs (numpy) and
returns the FULL output.  Internally shards data-parallel over batch across
8 NeuronCores (2 samples per core).

v4 design (v1 baseline 548us -> v2 411us -> v4):
  - PE is the roofline engine (~200K cycles/sample).  The enemy is the HAM
    clock gate: any recurring PE micro-stall (waiting on exp or PSUM-slot
    reuse) re-throttles the PE to 1.2 GHz.  So the attention inner loop is
    paced such that exp always finishes well before its PSUM slot is needed:
    attention(sample0) is INTERLEAVED with qkv/vT(sample1) at head-pair
    granularity, and attention(sample1) with proj(sample0).  The extra PE
    work between slot reuses (~2x) gives the exp engines ample slack.
  - exp split per m-chunk: one fused [128,1024] DVE Schraudolph fast-exp
    (tensor_scalar f32->int16 round-to-nearest = bf16 bits of exp; max elem
    err ~3.3%, cancelled by softmax normalization) + one fused ScalarE
    exact Exp.  Z rides along as a ones-column in the v^T stationary.
  - 1/Z: Z rows copied out of PSUM on ScalarE, DMA-packed to [16,128],
    one DVE reciprocal_approx_fast per head-pair, DMA-flattened, gpsimd
    partition_broadcast, DVE multiply.
  - qkv bias+bf16 cast: single fused ScalarE Identity per output chunk.
    proj bias pre-folded into the residual source tiles (DVE, gn phase).
"""

import math
import os

import numpy as np

import concourse.bacc as bacc
import concourse.tile as tile
from concourse import mybir
from concourse.bass_utils import run_bass_kernel_spmd

F32 = mybir.dt.float32
I16 = mybir.dt.int16
ALU = mybir.AluOpType
ACT = mybir.ActivationFunctionType

N_CORES = 8
B, C, HH, WW = 16, 512, 32, 32
L = HH * WW            # 1024
BL = B // N_CORES      # batches per core = 2
NH = 8                 # heads
CH = C // NH           # head dim = 64
GROUPS = 32
GS = C // GROUPS       # channels per group = 16
EPS = 1e-5
SCALE2 = 1.0 / math.sqrt(CH)   # combined q*k scale, folded into exp
CT = C // 128          # channel tiles = 4
ST = L // 128          # s-chunks = 8
INV_N = 1.0 / (GS * L)         # group mean divisor
# Schraudolph fast-exp constants (bf16-bit-space, round-to-nearest):
#   i16 = round(p * EXP_A + EXP_B);  bf16_bits(i16) ~= exp(SCALE2 * p)
EXP_A = SCALE2 * math.log2(math.e) * 128.0   # 23.083120654223414
EXP_B = (127.0 - 0.044) * 128.0              # 16250.368

MM_DT = mybir.dt.bfloat16

LAST_RESULTS = None  # test harness can read exec_time_ns from here


def _build_program():
    nc = bacc.Bacc("TRN2", target_bir_lowering=False, debug=False,
                   num_devices=N_CORES)

    x_d = nc.dram_tensor("x", [BL, C, L], F32, kind="ExternalInput").ap()
    out_d = nc.dram_tensor("out", [BL, C, L], F32, kind="ExternalOutput").ap()
    wqT_d = nc.dram_tensor("wqkvT", [C, 3 * C], MM_DT, kind="ExternalInput").ap()
    wpT_d = nc.dram_tensor("wprojT", [C, C], MM_DT, kind="ExternalInput").ap()
    nw_d = nc.dram_tensor("norm_w", [C], F32, kind="ExternalInput").ap()
    nb_d = nc.dram_tensor("norm_b", [C], F32, kind="ExternalInput").ap()
    qb_d = nc.dram_tensor("qkv_b", [3 * C], F32, kind="ExternalInput").ap()
    pb_d = nc.dram_tensor("proj_b", [C], F32, kind="ExternalInput").ap()
    sel_d = nc.dram_tensor("sel", [CT, 128, GROUPS], F32, kind="ExternalInput").ap()
    fan_d = nc.dram_tensor("fan", [CT, GROUPS, 128], F32, kind="ExternalInput").ap()
    ones_d = nc.dram_tensor("ones", [1, 512], MM_DT, kind="ExternalInput").ap()

    with tile.TileContext(nc) as tc:
        with (
            tc.tile_pool(name="wgt", bufs=1) as wgt,
            tc.tile_pool(name="xs", bufs=2 * CT) as xs_p,
            tc.tile_pool(name="xn", bufs=2 * CT) as xn_p,
            tc.tile_pool(name="qk", bufs=4 * CT) as qk_p,
            tc.tile_pool(name="vt", bufs=2 * ST) as vt_p,
            tc.tile_pool(name="ew", bufs=6) as ew_p,
            tc.tile_pool(name="apool", bufs=2 * CT) as a_p,
            tc.tile_pool(name="zz", bufs=2) as z_p,
            tc.tile_pool(name="zb", bufs=2) as zb_p,
            tc.tile_pool(name="outs", bufs=2) as out_p,
            tc.tile_pool(name="tiny", bufs=16) as tiny,
            tc.tile_pool(name="scr", bufs=4) as scr_p,
            tc.tile_pool(name="pmm", bufs=2, space="PSUM") as pmm,   # 2x 2-bank
            tc.tile_pool(name="pa", bufs=2, space="PSUM") as pa_p,   # 2x 2-bank
        ):
            # ---------------- constants / weights ----------------
            wq, wp, sel, fan, nw, nb = [], [], [], [], [], []
            for i in range(CT):
                w = wgt.tile([128, 3 * C], MM_DT, tag=f"wq{i}")
                nc.sync.dma_start(w[:], wqT_d[128 * i:128 * (i + 1), :])
                wq.append(w)
                w = wgt.tile([128, C], MM_DT, tag=f"wp{i}")
                nc.sync.dma_start(w[:], wpT_d[128 * i:128 * (i + 1), :])
                wp.append(w)
                s = wgt.tile([128, GROUPS], F32, tag=f"sel{i}")
                nc.sync.dma_start(s[:], sel_d[i, :, :])
                sel.append(s)
                f = wgt.tile([GROUPS, 128], F32, tag=f"fan{i}")
                nc.sync.dma_start(f[:], fan_d[i, :, :])
                fan.append(f)
                t = wgt.tile([128, 1], F32, tag=f"nw{i}")
                nc.sync.dma_start(t[:], nw_d[128 * i:128 * (i + 1)].rearrange("(p a) -> p a", a=1))
                nw.append(t)
                t = wgt.tile([128, 1], F32, tag=f"nb{i}")
                nc.sync.dma_start(t[:], nb_d[128 * i:128 * (i + 1)].rearrange("(p a) -> p a", a=1))
                nb.append(t)
            ones_t = wgt.tile([1, 512], MM_DT, tag="ones")
            nc.sync.dma_start(ones_t[:], ones_d[:, :])
            qbv_t = wgt.tile([1, C], MM_DT, tag="qbv")
            qbv_f = wgt.tile([1, C], F32, tag="qbvf")
            nc.sync.dma_start(qbv_f[:], qb_d[2 * C:3 * C].rearrange("(a b) -> a b", a=1))
            nc.vector.tensor_copy(qbv_t[:], qbv_f[:])
            qb_qk = []
            for j in range(2 * CT):
                t = wgt.tile([128, 1], F32, tag=f"qb{j}")
                nc.sync.dma_start(t[:], qb_d[128 * j:128 * (j + 1)].rearrange("(p a) -> p a", a=1))
                qb_qk.append(t)
            pb_col = []
            for j in range(CT):
                t = wgt.tile([128, 1], F32, tag=f"pbc{j}")
                nc.sync.dma_start(t[:], pb_d[128 * j:128 * (j + 1)].rearrange("(p a) -> p a", a=1))
                pb_col.append(t)
            eps_t = wgt.tile([GROUPS, 1], F32, tag="eps")
            nc.gpsimd.memset(eps_t[:], EPS)

            # ---- load x for both samples up front ----
            xs = {}
            for b in range(BL):
                for i in range(CT):
                    t = xs_p.tile([128, L], F32, tag="xs", name=f"xs{b}_{i}")
                    nc.sync.dma_start(t[:], x_d[b, 128 * i:128 * (i + 1), :])
                    xs[(b, i)] = t

            xn = {}
            qk = {}
            vt = {}
            a_tiles = {}

            def gn(b):
                stats = []
                for i in range(CT):
                    bns = scr_p.tile([128, 2, 6], F32, tag="bns", name=f"bns{b}_{i}")
                    xv = xs[(b, i)][:].rearrange("p (s f) -> p s f", f=512)
                    for sgi in range(2):
                        nc.vector.bn_stats(bns[:, sgi, :], xv[:, sgi, :])
                    mv = tiny.tile([128, 2], F32, tag="mv", name=f"mv{b}_{i}")
                    nc.vector.bn_aggr(mv[:], bns[:])
                    st = tiny.tile([128, 2], F32, tag="stats", name=f"st{b}_{i}")
                    nc.vector.tensor_scalar_mul(st[:, 0:1], mv[:, 0:1], float(L))
                    m2 = tiny.tile([128, 2], F32, tag="m2", name=f"m2{b}_{i}")
                    nc.vector.tensor_mul(m2[:, 0:1], mv[:, 0:1], mv[:, 0:1])
                    nc.vector.tensor_add(m2[:, 1:2], mv[:, 1:2], m2[:, 0:1])
                    nc.vector.tensor_scalar_mul(st[:, 1:2], m2[:, 1:2], float(L))
                    stats.append(st)
                pg = pmm.tile([128, 1024], F32, tag="mm", name=f"pg{b}")
                for i in range(CT):
                    nc.tensor.matmul(pg[0:GROUPS, 0:2], sel[i][:, :], stats[i][:, :],
                                     start=(i == 0), stop=(i == CT - 1))
                gq = tiny.tile([GROUPS, 8], F32, tag="gq", name=f"gq{b}")
                nc.vector.tensor_scalar_mul(gq[:, 0:1], pg[0:GROUPS, 0:1], INV_N)
                nc.vector.tensor_scalar_mul(gq[:, 1:2], pg[0:GROUPS, 1:2], INV_N)
                nc.vector.tensor_mul(gq[:, 2:3], gq[:, 0:1], gq[:, 0:1])
                nc.vector.tensor_sub(gq[:, 3:4], gq[:, 1:2], gq[:, 2:3])
                nc.scalar.activation(gq[:, 4:5], gq[:, 3:4], ACT.Sqrt,
                                     bias=eps_t[:])
                nc.vector.reciprocal(gq[:, 5:6], gq[:, 4:5])
                nc.vector.tensor_mul(gq[:, 6:7], gq[:, 0:1], gq[:, 5:6])
                for i in range(CT):
                    pf = pmm.tile([128, 1024], F32, tag="mm", name=f"pf{b}_{i}")
                    nc.tensor.matmul(pf[0:128, 0:2], fan[i][:, :], gq[:, 5:7],
                                     start=True, stop=True)
                    scb = tiny.tile([128, 3], F32, tag="scb", name=f"scb{b}_{i}")
                    nc.vector.tensor_mul(scb[:, 0:1], pf[0:128, 0:1], nw[i][:])
                    nc.vector.tensor_mul(scb[:, 1:2], pf[0:128, 1:2], nw[i][:])
                    nc.vector.tensor_sub(scb[:, 2:3], nb[i][:], scb[:, 1:2])
                    t = xn_p.tile([128, L], MM_DT, tag="xn", name=f"xn{b}_{i}")
                    nc.vector.tensor_scalar(t[:], xs[(b, i)][:], scb[:, 0:1],
                                            scb[:, 2:3], op0=ALU.mult, op1=ALU.add)
                    xn[(b, i)] = t
                    # fold proj bias into the residual source (in place)
                    nc.vector.tensor_scalar_add(xs[(b, i)][:], xs[(b, i)][:],
                                                pb_col[i][:])

            def qkv_j(b, j):
                t = qk_p.tile([128, L], MM_DT, tag="qk", name=f"qk{b}_{j}")
                pq = pmm.tile([128, 1024], F32, tag="mm", name=f"pq{b}_{j}")
                for n in range(2):
                    nsl = slice(512 * n, 512 * (n + 1))
                    for i in range(CT):
                        nc.tensor.matmul(pq[:, nsl],
                                         wq[i][:, 128 * j:128 * (j + 1)],
                                         xn[(b, i)][:, nsl],
                                         start=(i == 0), stop=(i == CT - 1))
                nc.scalar.activation(t[:, :], pq[:, :], ACT.Identity,
                                     bias=qb_qk[j][:])
                qk[(b, j)] = t

            def vt_m(b, m):
                pvf = pmm.tile([128, 1024], F32, tag="mm", name=f"pv{b}_{m}")
                pv = pvf[:, 0:512]
                for i in range(CT):
                    nc.tensor.matmul(pv,
                                     xn[(b, i)][:, 128 * m:128 * (m + 1)],
                                     wq[i][:, 2 * C:3 * C],
                                     start=(i == 0), stop=False)
                nc.tensor.matmul(pv, ones_t[0:1, 0:128], qbv_t[0:1, :],
                                 start=False, stop=True)
                t = vt_p.tile([128, NH * (CH + 1)], MM_DT, tag="vt",
                              name=f"vt{b}_{m}")
                dst = t[:].rearrange("p (h x) -> p h x", x=CH + 1)
                nc.scalar.activation(dst[:, :, 0:CH],
                                     pv.rearrange("p (h x) -> p h x", x=CH),
                                     ACT.Copy)
                nc.gpsimd.memset(dst[:, :, CH:CH + 1], 1.0)
                vt[(b, m)] = t

            def attn_hp(b, hp):
                q_t = qk[(b, hp)]
                k_t = qk[(b, CT + hp)]
                pa_h = [pa_p.tile([CH + 1, L], F32, tag="pa",
                                  name=f"pa{b}_{hp}_{hh}") for hh in range(2)]
                prev = None
                for m in range(ST):
                    msl = slice(128 * m, 128 * (m + 1))
                    pw = []
                    for hh, (plo, phi, tp) in enumerate(
                            ((0, CH, (0, 0)), (CH, 128, (CH, 0)))):
                        p = pmm.tile([128, 1024], F32, tag="mm",
                                     name=f"ps{b}_{hp}_{m}_{hh}")
                        for n in range(2):
                            nsl = slice(512 * n, 512 * (n + 1))
                            nc.tensor.matmul(p[:, nsl], k_t[plo:phi, msl],
                                             q_t[plo:phi, nsl],
                                             start=True, stop=True,
                                             tile_position=tp)
                        pw.append((hh, p))
                    ews = []
                    for hh, p in pw:
                        if (hh + m) % 2 == 0:
                            e = ew_p.tile([128, L], I16, tag="ewi")
                            nc.vector.tensor_scalar(e[:], p[:, :], EXP_A, EXP_B,
                                                    op0=ALU.mult, op1=ALU.add)
                            ews.append((hh, e.bitcast(MM_DT)))
                        else:
                            e = ew_p.tile([128, L], MM_DT, tag="ewa")
                            nc.scalar.activation(e[:], p[:, :], ACT.Exp,
                                                 scale=SCALE2)
                            ews.append((hh, e[:]))
                    if prev is not None:
                        pm, pews = prev
                        for hh, e in pews:
                            lhs = vt[(b, pm)][:, (2 * hp + hh) * (CH + 1):
                                              (2 * hp + hh + 1) * (CH + 1)]
                            for n in range(2):
                                nsl = slice(512 * n, 512 * (n + 1))
                                nc.tensor.matmul(pa_h[hh][:, nsl], lhs, e[:, nsl],
                                                 start=(pm == 0), stop=False)
                    prev = (m, ews)
                pm, pews = prev
                for hh, e in pews:
                    lhs = vt[(b, pm)][:, (2 * hp + hh) * (CH + 1):
                                      (2 * hp + hh + 1) * (CH + 1)]
                    for n in range(2):
                        nsl = slice(512 * n, 512 * (n + 1))
                        nc.tensor.matmul(pa_h[hh][:, nsl], lhs, e[:, nsl],
                                         start=False, stop=True)

                # ---- normalize ----
                zrow = []
                for hh in range(2):
                    zr = z_p.tile([1, L], F32, tag="zrow",
                                  name=f"zrow{b}_{hp}_{hh}")
                    nc.scalar.activation(zr[:, :], pa_h[hh][CH:CH + 1, :],
                                         ACT.Copy)
                    zrow.append(zr)
                zp = z_p.tile([16, 128], F32, tag="zp", name=f"zp{b}_{hp}")
                nc.sync.dma_start(zp[0:8, :], zrow[0][0:1, :])
                nc.sync.dma_start(zp[8:16, :], zrow[1][0:1, :])
                zrp = z_p.tile([16, 128], F32, tag="zrp", name=f"zrp{b}_{hp}")
                nc.vector.reciprocal_approx_fast(zrp[:], zp[:])
                a_t = a_p.tile([128, L], MM_DT, tag="a", name=f"a{b}_{hp}")
                for hh in range(2):
                    rzf = z_p.tile([1, L], F32, tag="rzf",
                                   name=f"rzf{b}_{hp}_{hh}")
                    nc.sync.dma_start(rzf[0:1, :], zrp[8 * hh:8 * (hh + 1), :])
                    rzb = zb_p.tile([CH, L], F32, tag="zb")
                    nc.gpsimd.partition_broadcast(rzb[:], rzf[:])
                    rows = slice(CH * hh, CH * (hh + 1))
                    nc.vector.tensor_mul(a_t[rows, :], pa_h[hh][0:CH, :],
                                         rzb[:, :])
                a_tiles[(b, hp)] = a_t

            def proj_j(b, j):
                o_t = out_p.tile([128, L], F32, tag="o", name=f"o{b}_{j}")
                pp = pmm.tile([128, 1024], F32, tag="mm", name=f"pp{b}_{j}")
                for n in range(2):
                    nsl = slice(512 * n, 512 * (n + 1))
                    for i in range(CT):
                        nc.tensor.matmul(pp[:, nsl],
                                         wp[i][:, 128 * j:128 * (j + 1)],
                                         a_tiles[(b, i)][:, nsl],
                                         start=(i == 0), stop=(i == CT - 1))
                nc.vector.tensor_add(o_t[:, :], pp[:, :], xs[(b, j)][:, :])
                nc.sync.dma_start(out_d[b, 128 * j:128 * (j + 1), :], o_t[:])

            # ---------------- schedule ----------------
            gn(0)
            gn(1)
            for j in range(2 * CT):
                qkv_j(0, j)
            for m in range(ST):
                vt_m(0, m)
            # attention(0) interleaved with qkv/vT of sample 1
            fill1 = ([("qkv", 1, j) for j in range(2 * CT)]
                     + [("vt", 1, m) for m in range(ST)])
            for hp in range(NH // 2):
                attn_hp(0, hp)
                for _ in range(4):
                    kind, bb, idx = fill1.pop(0)
                    (qkv_j if kind == "qkv" else vt_m)(bb, idx)
            # attention(1) interleaved with proj of sample 0
            for hp in range(NH // 2):
                attn_hp(1, hp)
                proj_j(0, hp)
            for j in range(CT):
                proj_j(1, j)

    nc.compile()
    return nc


_prog_cache = {}


def _get_program():
    if "p" not in _prog_cache:
        _prog_cache["p"] = _build_program()
    return _prog_cache["p"]


def _host_constants():
    sel = np.zeros((CT, 128, GROUPS), dtype=np.float32)
    fan = np.zeros((CT, GROUPS, 128), dtype=np.float32)
    for i in range(CT):
        for p in range(128):
            g = (128 * i + p) // GS
            sel[i, p, g] = 1.0
            fan[i, g, p] = 1.0
    ones = np.ones((1, 512), dtype=mybir.dt.np(MM_DT))
    return sel, fan, ones


def kernel(x, norm_w, norm_b, qkv_w, qkv_b, proj_w, proj_b):
    global LAST_RESULTS
    x = np.ascontiguousarray(np.asarray(x, dtype=np.float32))
    np_mm = mybir.dt.np(MM_DT)
    wqkvT = np.ascontiguousarray(np.asarray(qkv_w, dtype=np.float32).T.astype(np_mm))
    wprojT = np.ascontiguousarray(np.asarray(proj_w, dtype=np.float32).T.astype(np_mm))
    sel, fan, ones = _host_constants()

    xr = x.reshape(B, C, L)
    nc = _get_program()

    common = {
        "wqkvT": wqkvT,
        "wprojT": wprojT,
        "norm_w": np.ascontiguousarray(norm_w, dtype=np.float32),
        "norm_b": np.ascontiguousarray(norm_b, dtype=np.float32),
        "qkv_b": np.ascontiguousarray(qkv_b, dtype=np.float32),
        "proj_b": np.ascontiguousarray(proj_b, dtype=np.float32),
        "sel": sel,
        "fan": fan,
        "ones": ones,
    }
    in_maps = []
    for c in range(N_CORES):
        m = dict(common)
        m["x"] = np.ascontiguousarray(xr[BL * c:BL * (c + 1)])
        in_maps.append(m)

    trace = os.environ.get("KERNEL_TRACE", "0") == "1"
    kwargs = {}
    if trace:
        kwargs = dict(trace=True, trace_cores=[0])
    res = run_bass_kernel_spmd(nc, in_maps, core_ids=list(range(N_CORES)),
                               **kwargs)
    LAST_RESULTS = res
    out = np.concatenate([res.results[c]["out"] for c in range(N_CORES)], axis=0)
    return out.reshape(B, C, HH, WW)


# revision 12
# speedup vs baseline: 1.4028x; 1.0055x over previous
"""Trainium2 Bass kernel for nn_AttentionBlock (GroupNorm + 1x1-conv QKV +
multi-head attention + 1x1-conv proj + residual).

Contract: kernel(**inputs) takes the FULL unsharded inputs (numpy) and
returns the FULL output.  Internally shards data-parallel over batch across
8 NeuronCores (2 samples per core).

v4 design (v1 baseline 548us -> v2 411us -> v4):
  - PE is the roofline engine (~200K cycles/sample).  The enemy is the HAM
    clock gate: any recurring PE micro-stall (waiting on exp or PSUM-slot
    reuse) re-throttles the PE to 1.2 GHz.  So the attention inner loop is
    paced such that exp always finishes well before its PSUM slot is needed:
    attention(sample0) is INTERLEAVED with qkv/vT(sample1) at head-pair
    granularity, and attention(sample1) with proj(sample0).  The extra PE
    work between slot reuses (~2x) gives the exp engines ample slack.
  - exp split per m-chunk: one fused [128,1024] DVE Schraudolph fast-exp
    (tensor_scalar f32->int16 round-to-nearest = bf16 bits of exp; max elem
    err ~3.3%, cancelled by softmax normalization) + one fused ScalarE
    exact Exp.  Z rides along as a ones-column in the v^T stationary.
  - 1/Z: Z rows copied out of PSUM on ScalarE, DMA-packed to [16,128],
    one DVE reciprocal_approx_fast per head-pair, DMA-flattened, gpsimd
    partition_broadcast, DVE multiply.
  - qkv bias+bf16 cast: single fused ScalarE Identity per output chunk.
    proj bias pre-folded into the residual source tiles (DVE, gn phase).
"""

import math
import os

import numpy as np

import concourse.bacc as bacc
import concourse.tile as tile
from concourse import mybir
from concourse.bass_utils import run_bass_kernel_spmd

F32 = mybir.dt.float32
I16 = mybir.dt.int16
ALU = mybir.AluOpType
ACT = mybir.ActivationFunctionType

N_CORES = 8
B, C, HH, WW = 16, 512, 32, 32
L = HH * WW            # 1024
BL = B // N_CORES      # batches per core = 2
NH = 8                 # heads
CH = C // NH           # head dim = 64
GROUPS = 32
GS = C // GROUPS       # channels per group = 16
EPS = 1e-5
SCALE2 = 1.0 / math.sqrt(CH)   # combined q*k scale, folded into exp
CT = C // 128          # channel tiles = 4
ST = L // 128          # s-chunks = 8
INV_N = 1.0 / (GS * L)         # group mean divisor
# Schraudolph fast-exp constants (bf16-bit-space, round-to-nearest):
#   i16 = round(p * EXP_A + EXP_B);  bf16_bits(i16) ~= exp(SCALE2 * p)
EXP_A = SCALE2 * math.log2(math.e) * 128.0   # 23.083120654223414
EXP_B = (127.0 - 0.044) * 128.0              # 16250.368

MM_DT = mybir.dt.bfloat16

LAST_RESULTS = None  # test harness can read exec_time_ns from here


def _build_program():
    nc = bacc.Bacc("TRN2", target_bir_lowering=False, debug=False,
                   num_devices=N_CORES)

    x_d = nc.dram_tensor("x", [BL, C, L], F32, kind="ExternalInput").ap()
    out_d = nc.dram_tensor("out", [BL, C, L], F32, kind="ExternalOutput").ap()
    wqT_d = nc.dram_tensor("wqkvT", [C, 3 * C], MM_DT, kind="ExternalInput").ap()
    wpT_d = nc.dram_tensor("wprojT", [C, C], MM_DT, kind="ExternalInput").ap()
    nw_d = nc.dram_tensor("norm_w", [C], F32, kind="ExternalInput").ap()
    nb_d = nc.dram_tensor("norm_b", [C], F32, kind="ExternalInput").ap()
    qb_d = nc.dram_tensor("qkv_b", [3 * C], F32, kind="ExternalInput").ap()
    pb_d = nc.dram_tensor("proj_b", [C], F32, kind="ExternalInput").ap()
    sel_d = nc.dram_tensor("sel", [CT, 128, GROUPS], F32, kind="ExternalInput").ap()
    fan_d = nc.dram_tensor("fan", [CT, GROUPS, 128], F32, kind="ExternalInput").ap()
    ones_d = nc.dram_tensor("ones", [1, 512], MM_DT, kind="ExternalInput").ap()

    with tile.TileContext(nc) as tc:
        with (
            tc.tile_pool(name="wgt", bufs=1) as wgt,
            tc.tile_pool(name="xs", bufs=2 * CT) as xs_p,
            tc.tile_pool(name="xn", bufs=2 * CT) as xn_p,
            tc.tile_pool(name="qk", bufs=4 * CT) as qk_p,
            tc.tile_pool(name="vt", bufs=2 * ST) as vt_p,
            tc.tile_pool(name="ew", bufs=6) as ew_p,
            tc.tile_pool(name="apool", bufs=2 * CT) as a_p,
            tc.tile_pool(name="zz", bufs=2) as z_p,
            tc.tile_pool(name="zb", bufs=2) as zb_p,
            tc.tile_pool(name="outs", bufs=2) as out_p,
            tc.tile_pool(name="tiny", bufs=16) as tiny,
            tc.tile_pool(name="scr", bufs=4) as scr_p,
            tc.tile_pool(name="pmm", bufs=2, space="PSUM") as pmm,   # 2x 2-bank
            tc.tile_pool(name="pa", bufs=2, space="PSUM") as pa_p,   # 2x 2-bank
        ):
            # ---------------- constants / weights ----------------
            wq, wp, sel, fan, nw, nb = [], [], [], [], [], []
            for i in range(CT):
                w = wgt.tile([128, 3 * C], MM_DT, tag=f"wq{i}")
                nc.sync.dma_start(w[:], wqT_d[128 * i:128 * (i + 1), :])
                wq.append(w)
                w = wgt.tile([128, C], MM_DT, tag=f"wp{i}")
                nc.sync.dma_start(w[:], wpT_d[128 * i:128 * (i + 1), :])
                wp.append(w)
                s = wgt.tile([128, GROUPS], F32, tag=f"sel{i}")
                nc.sync.dma_start(s[:], sel_d[i, :, :])
                sel.append(s)
                f = wgt.tile([GROUPS, 128], F32, tag=f"fan{i}")
                nc.sync.dma_start(f[:], fan_d[i, :, :])
                fan.append(f)
                t = wgt.tile([128, 1], F32, tag=f"nw{i}")
                nc.sync.dma_start(t[:], nw_d[128 * i:128 * (i + 1)].rearrange("(p a) -> p a", a=1))
                nw.append(t)
                t = wgt.tile([128, 1], F32, tag=f"nb{i}")
                nc.sync.dma_start(t[:], nb_d[128 * i:128 * (i + 1)].rearrange("(p a) -> p a", a=1))
                nb.append(t)
            ones_t = wgt.tile([1, 512], MM_DT, tag="ones")
            nc.sync.dma_start(ones_t[:], ones_d[:, :])
            qbv_t = wgt.tile([1, C], MM_DT, tag="qbv")
            qbv_f = wgt.tile([1, C], F32, tag="qbvf")
            nc.sync.dma_start(qbv_f[:], qb_d[2 * C:3 * C].rearrange("(a b) -> a b", a=1))
            nc.vector.tensor_copy(qbv_t[:], qbv_f[:])
            qb_qk = []
            for j in range(2 * CT):
                t = wgt.tile([128, 1], F32, tag=f"qb{j}")
                nc.sync.dma_start(t[:], qb_d[128 * j:128 * (j + 1)].rearrange("(p a) -> p a", a=1))
                qb_qk.append(t)
            pb_col = []
            for j in range(CT):
                t = wgt.tile([128, 1], F32, tag=f"pbc{j}")
                nc.sync.dma_start(t[:], pb_d[128 * j:128 * (j + 1)].rearrange("(p a) -> p a", a=1))
                pb_col.append(t)
            eps_t = wgt.tile([GROUPS, 1], F32, tag="eps")
            nc.gpsimd.memset(eps_t[:], EPS)

            # ---- load x for both samples up front ----
            xs = {}
            for b in range(BL):
                for i in range(CT):
                    t = xs_p.tile([128, L], F32, tag="xs", name=f"xs{b}_{i}")
                    nc.sync.dma_start(t[:], x_d[b, 128 * i:128 * (i + 1), :])
                    xs[(b, i)] = t

            xn = {}
            qk = {}
            vt = {}
            a_tiles = {}

            def gn(b):
                stats = []
                for i in range(CT):
                    bns = scr_p.tile([128, 2, 6], F32, tag="bns", name=f"bns{b}_{i}")
                    xv = xs[(b, i)][:].rearrange("p (s f) -> p s f", f=512)
                    for sgi in range(2):
                        nc.vector.bn_stats(bns[:, sgi, :], xv[:, sgi, :])
                    mv = tiny.tile([128, 2], F32, tag="mv", name=f"mv{b}_{i}")
                    nc.vector.bn_aggr(mv[:], bns[:])
                    st = tiny.tile([128, 2], F32, tag="stats", name=f"st{b}_{i}")
                    nc.vector.tensor_scalar_mul(st[:, 0:1], mv[:, 0:1], float(L))
                    m2 = tiny.tile([128, 2], F32, tag="m2", name=f"m2{b}_{i}")
                    nc.vector.tensor_mul(m2[:, 0:1], mv[:, 0:1], mv[:, 0:1])
                    nc.vector.tensor_add(m2[:, 1:2], mv[:, 1:2], m2[:, 0:1])
                    nc.vector.tensor_scalar_mul(st[:, 1:2], m2[:, 1:2], float(L))
                    stats.append(st)
                pg = pmm.tile([128, 1024], F32, tag="mm", name=f"pg{b}")
                for i in range(CT):
                    nc.tensor.matmul(pg[0:GROUPS, 0:2], sel[i][:, :], stats[i][:, :],
                                     start=(i == 0), stop=(i == CT - 1))
                gq = tiny.tile([GROUPS, 8], F32, tag="gq", name=f"gq{b}")
                nc.vector.tensor_scalar_mul(gq[:, 0:1], pg[0:GROUPS, 0:1], INV_N)
                nc.vector.tensor_scalar_mul(gq[:, 1:2], pg[0:GROUPS, 1:2], INV_N)
                nc.vector.tensor_mul(gq[:, 2:3], gq[:, 0:1], gq[:, 0:1])
                nc.vector.tensor_sub(gq[:, 3:4], gq[:, 1:2], gq[:, 2:3])
                nc.scalar.activation(gq[:, 4:5], gq[:, 3:4], ACT.Sqrt,
                                     bias=eps_t[:])
                nc.vector.reciprocal(gq[:, 5:6], gq[:, 4:5])
                nc.vector.tensor_mul(gq[:, 6:7], gq[:, 0:1], gq[:, 5:6])
                for i in range(CT):
                    pf = pmm.tile([128, 1024], F32, tag="mm", name=f"pf{b}_{i}")
                    nc.tensor.matmul(pf[0:128, 0:2], fan[i][:, :], gq[:, 5:7],
                                     start=True, stop=True)
                    scb = tiny.tile([128, 3], F32, tag="scb", name=f"scb{b}_{i}")
                    nc.vector.tensor_mul(scb[:, 0:1], pf[0:128, 0:1], nw[i][:])
                    nc.vector.tensor_mul(scb[:, 1:2], pf[0:128, 1:2], nw[i][:])
                    nc.vector.tensor_sub(scb[:, 2:3], nb[i][:], scb[:, 1:2])
                    t = xn_p.tile([128, L], MM_DT, tag="xn", name=f"xn{b}_{i}")
                    nc.vector.tensor_scalar(t[:], xs[(b, i)][:], scb[:, 0:1],
                                            scb[:, 2:3], op0=ALU.mult, op1=ALU.add)
                    xn[(b, i)] = t
                    # fold proj bias into the residual source (in place)
                    nc.vector.tensor_scalar_add(xs[(b, i)][:], xs[(b, i)][:],
                                                pb_col[i][:])

            def qkv_j(b, j):
                t = qk_p.tile([128, L], MM_DT, tag="qk", name=f"qk{b}_{j}")
                pq = pmm.tile([128, 1024], F32, tag="mm", name=f"pq{b}_{j}")
                for n in range(2):
                    nsl = slice(512 * n, 512 * (n + 1))
                    for i in range(CT):
                        nc.tensor.matmul(pq[:, nsl],
                                         wq[i][:, 128 * j:128 * (j + 1)],
                                         xn[(b, i)][:, nsl],
                                         start=(i == 0), stop=(i == CT - 1))
                nc.scalar.activation(t[:, :], pq[:, :], ACT.Identity,
                                     bias=qb_qk[j][:])
                qk[(b, j)] = t

            def vt_m(b, m):
                pvf = pmm.tile([128, 1024], F32, tag="mm", name=f"pv{b}_{m}")
                pv = pvf[:, 0:512]
                for i in range(CT):
                    nc.tensor.matmul(pv,
                                     xn[(b, i)][:, 128 * m:128 * (m + 1)],
                                     wq[i][:, 2 * C:3 * C],
                                     start=(i == 0), stop=False)
                nc.tensor.matmul(pv, ones_t[0:1, 0:128], qbv_t[0:1, :],
                                 start=False, stop=True)
                t = vt_p.tile([128, NH * (CH + 1)], MM_DT, tag="vt",
                              name=f"vt{b}_{m}")
                dst = t[:].rearrange("p (h x) -> p h x", x=CH + 1)
                nc.vector.tensor_copy(dst[:, :, 0:CH],
                                      pv.rearrange("p (h x) -> p h x", x=CH))
                nc.gpsimd.memset(dst[:, :, CH:CH + 1], 1.0)
                vt[(b, m)] = t

            def attn_hp(b, hp, fctx=None):
                q_t = qk[(b, hp)]
                k_t = qk[(b, CT + hp)]
                pa_h = [pa_p.tile([CH + 1, L], F32, tag="pa",
                                  name=f"pa{b}_{hp}_{hh}") for hh in range(2)]
                prev = None
                for m in range(ST):
                    msl = slice(128 * m, 128 * (m + 1))
                    pw = []
                    for hh, (plo, phi, tp) in enumerate(
                            ((0, CH, (0, 0)), (CH, 128, (CH, 0)))):
                        p = pmm.tile([128, 1024], F32, tag="mm",
                                     name=f"ps{b}_{hp}_{m}_{hh}")
                        for n in range(2):
                            nsl = slice(512 * n, 512 * (n + 1))
                            nc.tensor.matmul(p[:, nsl], k_t[plo:phi, msl],
                                             q_t[plo:phi, nsl],
                                             start=True, stop=True,
                                             tile_position=tp)
                        pw.append((hh, p))
                    ews = []
                    for hh, p in pw:
                        if (hh + m) % 2 == 0:
                            e = ew_p.tile([128, L], I16, tag="ewi")
                            nc.vector.tensor_scalar(e[:], p[:, :], EXP_A, EXP_B,
                                                    op0=ALU.mult, op1=ALU.add)
                            ews.append((hh, e.bitcast(MM_DT)))
                        else:
                            e = ew_p.tile([128, L], MM_DT, tag="ewa")
                            nc.scalar.activation(e[:], p[:, :], ACT.Exp,
                                                 scale=SCALE2)
                            ews.append((hh, e[:]))
                    if prev is not None:
                        pm, pews = prev
                        for hh, e in pews:
                            lhs = vt[(b, pm)][:, (2 * hp + hh) * (CH + 1):
                                              (2 * hp + hh + 1) * (CH + 1)]
                            for n in range(2):
                                nsl = slice(512 * n, 512 * (n + 1))
                                nc.tensor.matmul(pa_h[hh][:, nsl], lhs, e[:, nsl],
                                                 start=(pm == 0), stop=False)
                    prev = (m, ews)
                    # evenly-paced fill task between m-chunks keeps the PE fed
                    # while exp drains its PSUM slot
                    if fctx is not None and fctx["fills"]:
                        k = fctx["step"]
                        fctx["step"] += 1
                        if (k + 1) * fctx["U"] // fctx["S"] > k * fctx["U"] // fctx["S"]:
                            fctx["fills"].pop(0)()
                pm, pews = prev
                for hh, e in pews:
                    lhs = vt[(b, pm)][:, (2 * hp + hh) * (CH + 1):
                                      (2 * hp + hh + 1) * (CH + 1)]
                    for n in range(2):
                        nsl = slice(512 * n, 512 * (n + 1))
                        nc.tensor.matmul(pa_h[hh][:, nsl], lhs, e[:, nsl],
                                         start=False, stop=True)

                # ---- normalize ----
                zrow = []
                for hh in range(2):
                    zr = z_p.tile([1, L], F32, tag="zrow",
                                  name=f"zrow{b}_{hp}_{hh}")
                    nc.scalar.activation(zr[:, :], pa_h[hh][CH:CH + 1, :],
                                         ACT.Copy)
                    zrow.append(zr)
                zp = z_p.tile([16, 128], F32, tag="zp", name=f"zp{b}_{hp}")
                nc.sync.dma_start(zp[0:8, :], zrow[0][0:1, :])
                nc.sync.dma_start(zp[8:16, :], zrow[1][0:1, :])
                zrp = z_p.tile([16, 128], F32, tag="zrp", name=f"zrp{b}_{hp}")
                nc.vector.reciprocal_approx_fast(zrp[:], zp[:])
                a_t = a_p.tile([128, L], MM_DT, tag="a", name=f"a{b}_{hp}")
                for hh in range(2):
                    rzf = z_p.tile([1, L], F32, tag="rzf",
                                   name=f"rzf{b}_{hp}_{hh}")
                    nc.sync.dma_start(rzf[0:1, :], zrp[8 * hh:8 * (hh + 1), :])
                    rzb = zb_p.tile([CH, L], F32, tag="zb")
                    nc.gpsimd.partition_broadcast(rzb[:], rzf[:])
                    rows = slice(CH * hh, CH * (hh + 1))
                    nc.vector.tensor_mul(a_t[rows, :], pa_h[hh][0:CH, :],
                                         rzb[:, :])
                a_tiles[(b, hp)] = a_t

            def proj_j(b, j):
                o_t = out_p.tile([128, L], F32, tag="o", name=f"o{b}_{j}")
                pp = pmm.tile([128, 1024], F32, tag="mm", name=f"pp{b}_{j}")
                for n in range(2):
                    nsl = slice(512 * n, 512 * (n + 1))
                    for i in range(CT):
                        nc.tensor.matmul(pp[:, nsl],
                                         wp[i][:, 128 * j:128 * (j + 1)],
                                         a_tiles[(b, i)][:, nsl],
                                         start=(i == 0), stop=(i == CT - 1))
                nc.vector.tensor_add(o_t[:, :], pp[:, :], xs[(b, j)][:, :])
                nc.sync.dma_start(out_d[b, 128 * j:128 * (j + 1), :], o_t[:])

            # ---------------- schedule ----------------
            # PE warm-up: real-shaped dummy matmuls while DMA/stats run, so
            # the HAM un-throttles before qkv(0) begins
            wu = pmm.tile([128, 1024], F32, tag="mm", name="warm")
            for _ in range(18):
                nc.tensor.matmul(wu[:, 0:512], wq[0][:, 0:128],
                                 wq[0][:, 0:512].bitcast(MM_DT), start=True,
                                 stop=True)
            gn(0)
            gn(1)
            for j in range(2 * CT):
                qkv_j(0, j)
            for m in range(ST):
                vt_m(0, m)
            # attention(0) interleaved per-m with qkv/vT of sample 1
            fills1 = ([(lambda jj: (lambda: qkv_j(1, jj)))(j) for j in range(2 * CT)]
                      + [(lambda mm_: (lambda: vt_m(1, mm_)))(m) for m in range(ST)])
            fctx1 = {"fills": fills1, "step": 0, "U": len(fills1), "S": 32}
            for hp in range(NH // 2):
                attn_hp(0, hp, fctx1)
            while fctx1["fills"]:
                fctx1["fills"].pop(0)()
            # attention(1) interleaved per-m with proj of sample 0
            fills2 = [(lambda jj: (lambda: proj_j(0, jj)))(j) for j in range(CT)]
            fctx2 = {"fills": fills2, "step": 0, "U": len(fills2), "S": 34}
            for hp in range(NH // 2):
                attn_hp(1, hp, fctx2)
            while fctx2["fills"]:
                fctx2["fills"].pop(0)()
            for j in range(CT):
                proj_j(1, j)

    nc.compile()
    return nc


_prog_cache = {}


def _get_program():
    if "p" not in _prog_cache:
        _prog_cache["p"] = _build_program()
    return _prog_cache["p"]


def _host_constants():
    sel = np.zeros((CT, 128, GROUPS), dtype=np.float32)
    fan = np.zeros((CT, GROUPS, 128), dtype=np.float32)
    for i in range(CT):
        for p in range(128):
            g = (128 * i + p) // GS
            sel[i, p, g] = 1.0
            fan[i, g, p] = 1.0
    ones = np.ones((1, 512), dtype=mybir.dt.np(MM_DT))
    return sel, fan, ones


def kernel(x, norm_w, norm_b, qkv_w, qkv_b, proj_w, proj_b):
    global LAST_RESULTS
    x = np.ascontiguousarray(np.asarray(x, dtype=np.float32))
    np_mm = mybir.dt.np(MM_DT)
    wqkvT = np.ascontiguousarray(np.asarray(qkv_w, dtype=np.float32).T.astype(np_mm))
    wprojT = np.ascontiguousarray(np.asarray(proj_w, dtype=np.float32).T.astype(np_mm))
    sel, fan, ones = _host_constants()

    xr = x.reshape(B, C, L)
    nc = _get_program()

    common = {
        "wqkvT": wqkvT,
        "wprojT": wprojT,
        "norm_w": np.ascontiguousarray(norm_w, dtype=np.float32),
        "norm_b": np.ascontiguousarray(norm_b, dtype=np.float32),
        "qkv_b": np.ascontiguousarray(qkv_b, dtype=np.float32),
        "proj_b": np.ascontiguousarray(proj_b, dtype=np.float32),
        "sel": sel,
        "fan": fan,
        "ones": ones,
    }
    in_maps = []
    for c in range(N_CORES):
        m = dict(common)
        m["x"] = np.ascontiguousarray(xr[BL * c:BL * (c + 1)])
        in_maps.append(m)

    trace = os.environ.get("KERNEL_TRACE", "0") == "1"
    kwargs = {}
    if trace:
        kwargs = dict(trace=True, trace_cores=[0])
    res = run_bass_kernel_spmd(nc, in_maps, core_ids=list(range(N_CORES)),
                               **kwargs)
    LAST_RESULTS = res
    out = np.concatenate([res.results[c]["out"] for c in range(N_CORES)], axis=0)
    return out.reshape(B, C, HH, WW)
